# revision 2
# baseline (speedup 1.0000x reference)
"""Trainium2 Bass kernel for block-causal (frame) multi-head attention.

Model (reference returns a 5-tuple):
    qh, kh, vh = per-head projections        [1, 12, 3136, 64]
    attention  = softmaxed masked scores      [1, 12, 3136, 3136]
    out        = output projection            [1, 3136, 768]

Strategy (8 NeuronCores, SPMD, uniform program, per-core data):
  Phase 1  (row-sharded QKV projection): core c computes q/k/v projections
      for its 392 query rows (frames 2c, 2c+1) in transposed layout
      [768, 392] via float32r matmuls.  Host gathers.
  Phase 2  (query-sharded attention): core c handles query frames 2c,2c+1
      for all 12 heads over all (padded) 3200 keys.  Scores are computed
      k-major (keys on partitions) with the block-causal mask folded into
      the matmul as 2 extra contraction rows (indicator x -BIG block).
      exp on ACT (scale=1/8 folded), denominator via an appended ones
      column in V, AV accumulated in PSUM, normalize on DVE, attention
      written k-major (host assembles the transpose).  Output projection
      + bias on-device.
"""

import numpy as np

import concourse.bacc as bacc
import concourse.mybir as mybir
import concourse.tile as tile
from concourse.bass_utils import run_bass_kernel_spmd

H = 12
D = 768
DK = 64
NP = 196
NF = 16
S = 3136
P = 128
NCORE = 8
QW = S // NCORE          # 392 query rows per core
SP = 3200                # keys padded to 25*128
NCH = SP // P            # 25 key chunks
CD = DK + 2              # contraction rows incl. 2 mask rows
VW = DK + 1              # v width incl. ones column
BIGNEG = np.float32(-60000.0)

F32 = mybir.dt.float32
F32R = mybir.dt.float32r
AF = mybir.ActivationFunctionType
CORE_IDS = list(range(NCORE))

_NC_CACHE = {}


def _build_phase1():
    nc = bacc.Bacc("TRN2", target_bir_lowering=False)
    xT = nc.dram_tensor("xT", [3, P, 6, QW], F32, kind="ExternalInput")
    wT = nc.dram_tensor("wT", [3, P, 6, D], F32, kind="ExternalInput")
    b3 = nc.dram_tensor("b3", [P, 18], F32, kind="ExternalInput")
    qkvT = nc.dram_tensor("qkvT", [3, 6, P, QW], F32, kind="ExternalOutput")

    with tile.TileContext(nc) as tc:
        with (
            tc.tile_pool(name="wp", bufs=1) as wp,
            tc.tile_pool(name="xp", bufs=1) as xp,
            tc.tile_pool(name="bp", bufs=1) as bp,
            tc.tile_pool(name="op", bufs=4) as op,
            tc.tile_pool(name="ps", bufs=4, space="PSUM") as ps,
        ):
            bt = bp.tile([P, 18], F32)
            nc.sync.dma_start(bt[:], b3[:])
            wts, xts = [], []
            for t in range(3):
                w = wp.tile([P, 6, D], F32R, tag=f"w{t}")
                nc.sync.dma_start(w[:], wT[t].bitcast(F32R))
                wts.append(w)
                x = xp.tile([P, 6, QW], F32R, tag=f"x{t}")
                nc.sync.dma_start(x[:], xT[t].bitcast(F32R))
                xts.append(x)
            for t in range(3):
                for dc in range(6):
                    pt = ps.tile([P, QW], F32)
                    for ec in range(6):
                        nc.tensor.matmul(
                            pt[:],
                            wts[t][:, ec, dc * P : (dc + 1) * P],
                            xts[t][:, ec, :],
                            start=(ec == 0),
                            stop=(ec == 5),
                        )
                    ot = op.tile([P, QW], F32, tag="out")
                    nc.scalar.activation(
                        ot[:],
                        pt[:],
                        AF.Identity,
                        bias=bt[:, t * 6 + dc : t * 6 + dc + 1],
                    )
                    nc.sync.dma_start(qkvT[t, dc], ot[:])
    nc.compile()
    return nc


def _build_phase2():
    nc = bacc.Bacc("TRN2", target_bir_lowering=False)
    khT = nc.dram_tensor("khT", [H, DK, SP], F32, kind="ExternalInput")
    minds = nc.dram_tensor("minds", [2, SP], F32, kind="ExternalInput")
    qhT = nc.dram_tensor("qhT", [H, CD, QW], F32, kind="ExternalInput")
    vh = nc.dram_tensor("vh", [H, P, NCH * VW], F32, kind="ExternalInput")
    woT = nc.dram_tensor("woT", [P, 6, D], F32, kind="ExternalInput")
    bo = nc.dram_tensor("bo", [1, D], F32, kind="ExternalInput")
    ones = nc.dram_tensor("ones", [1, QW], F32, kind="ExternalInput")
    attn = nc.dram_tensor("attn", [H, SP, QW], F32, kind="ExternalOutput")
    outp = nc.dram_tensor("outp", [QW, D], F32, kind="ExternalOutput")

    with tile.TileContext(nc) as tc:
        with (
            tc.tile_pool(name="kp", bufs=2) as kp,
            tc.tile_pool(name="qp", bufs=2) as qp,
            tc.tile_pool(name="vp", bufs=2) as vp,
            tc.tile_pool(name="ep", bufs=26) as ep,
            tc.tile_pool(name="ap", bufs=6) as ap,
            tc.tile_pool(name="cp", bufs=1) as cp,
            tc.tile_pool(name="ctp", bufs=2) as ctp,
            tc.tile_pool(name="rp", bufs=2) as rp,
            tc.tile_pool(name="wp", bufs=1) as wp,
            tc.tile_pool(name="bp", bufs=1) as bp,
            tc.tile_pool(name="op", bufs=3) as op,
            tc.tile_pool(name="ps_s", bufs=3, space="PSUM") as ps_s,
            tc.tile_pool(name="ps_o", bufs=2, space="PSUM") as ps_o,
            tc.tile_pool(name="ps_b", bufs=1, space="PSUM") as ps_b,
            tc.tile_pool(name="ps_f", bufs=2, space="PSUM") as ps_f,
        ):
            wo_t = wp.tile([P, 6, D], F32R)
            nc.sync.dma_start(wo_t[:], woT[:].bitcast(F32R))
            bo_t = bp.tile([1, D], F32R, tag="bo")
            nc.sync.dma_start(bo_t[:], bo[:].bitcast(F32R))
            ones_f = bp.tile([1, QW], F32, tag="ones_f")
            nc.sync.dma_start(ones_f[:], ones[:])
            ones_r = bp.tile([1, QW], F32R, tag="ones_r")
            nc.sync.dma_start(ones_r[:], ones[:].bitcast(F32R))

            concatT = cp.tile([P, 6, QW], F32R)

            for h in range(H):
                ka = kp.tile([CD, SP], F32R, tag="ka")
                nc.sync.dma_start(ka[0:DK, :], khT[h].bitcast(F32R))
                nc.sync.dma_start(ka[DK:CD, :], minds[:].bitcast(F32R))
                qa = qp.tile([CD, QW], F32R, tag="qa")
                nc.sync.dma_start(qa[:], qhT[h].bitcast(F32R))
                va = vp.tile([P, NCH * VW], F32R, tag="va")
                nc.sync.dma_start(va[:], vh[h].bitcast(F32R))

                po = ps_o.tile([VW, QW], F32, tag="po")
                exps = []
                for j in range(NCH):
                    st = ps_s.tile([P, QW], F32, tag="st")
                    nc.tensor.matmul(
                        st[:], ka[:, j * P : (j + 1) * P], qa[:],
                        start=True, stop=True,
                    )
                    e = ep.tile([P, QW], F32R, tag="exp")
                    nc.scalar.activation(e[:], st[:], AF.Exp, scale=0.125)
                    exps.append(e)
                    nc.tensor.matmul(
                        po[:], va[:, j * VW : (j + 1) * VW], e[:],
                        start=(j == 0), stop=(j == NCH - 1),
                    )

                rc = rp.tile([1, QW], F32, tag="rc")
                nc.vector.reciprocal(rc[:], po[DK : DK + 1, :])
                pb = ps_b.tile([P, QW], F32, tag="pb")
                nc.tensor.matmul(pb[:], ones_f[:, 0:P], rc[:], start=True, stop=True)
                rb = rp.tile([P, QW], F32, tag="rb")
                nc.scalar.copy(rb[:], pb[:])

                for j in range(NCH):
                    at = ap.tile([P, QW], F32, tag="attn")
                    nc.vector.tensor_tensor(
                        at[:], exps[j][:].bitcast(F32), rb[:], mybir.AluOpType.mult
                    )
                    nc.sync.dma_start(attn[h, j * P : (j + 1) * P, :], at[:])

                cs = ctp.tile([DK, QW], F32, tag="cs")
                nc.vector.tensor_tensor(
                    cs[:], po[0:DK, :], rb[0:DK, :], mybir.AluOpType.mult
                )
                nc.scalar.copy(
                    concatT[(h % 2) * DK : (h % 2) * DK + DK, h // 2, :], cs[:]
                )

            for qc in range(4):
                outs = op.tile([98, D], F32, tag="outs")
                for half in range(2):
                    pf = ps_f.tile([98, 384], F32, tag="pf")
                    for ec in range(6):
                        nc.tensor.matmul(
                            pf[:],
                            concatT[:, ec, qc * 98 : (qc + 1) * 98],
                            wo_t[:, ec, half * 384 : (half + 1) * 384],
                            start=(ec == 0),
                            stop=False,
                            skip_group_check=True,
                        )
                    nc.tensor.matmul(
                        pf[:],
                        ones_r[:, qc * 98 : (qc + 1) * 98],
                        bo_t[:, half * 384 : (half + 1) * 384],
                        start=False,
                        stop=True,
                        skip_group_check=True,
                    )
                    nc.scalar.copy(outs[:, half * 384 : (half + 1) * 384], pf[:])
                nc.sync.dma_start(outp[qc * 98 : (qc + 1) * 98, :], outs[:])
    nc.compile()
    return nc


def _get_ncs():
    if "nc1" not in _NC_CACHE:
        _NC_CACHE["nc1"] = _build_phase1()
        _NC_CACHE["nc2"] = _build_phase2()
    return _NC_CACHE["nc1"], _NC_CACHE["nc2"]


def kernel(query, key, value, Wq, bq, Wk, bk, Wv, bv, Wo, bo):
    f32 = np.float32
    query = np.ascontiguousarray(np.asarray(query, f32))
    key = np.ascontiguousarray(np.asarray(key, f32))
    value = np.ascontiguousarray(np.asarray(value, f32))
    Wq, Wk, Wv, Wo = (np.asarray(w, f32) for w in (Wq, Wk, Wv, Wo))
    bq, bk, bv, bo = (np.asarray(b, f32) for b in (bq, bk, bv, bo))

    nc1, nc2 = _get_ncs()

    # ---------------- phase 1: QKV projection (row-sharded) ----------------
    wT_h = np.ascontiguousarray(
        np.stack([Wq.T, Wk.T, Wv.T]).reshape(3, 6, P, D).transpose(0, 2, 1, 3)
    )
    b3_h = np.ascontiguousarray(
        np.stack([bq, bk, bv]).reshape(3, 6, P).transpose(2, 0, 1).reshape(P, 18)
    )
    in_maps1 = []
    for c in range(NCORE):
        rows = slice(QW * c, QW * (c + 1))
        X = np.stack([query[0, rows], key[0, rows], value[0, rows]])
        xT_h = np.ascontiguousarray(
            X.transpose(0, 2, 1).reshape(3, 6, P, QW).transpose(0, 2, 1, 3)
        )
        in_maps1.append({"xT": xT_h, "wT": wT_h, "b3": b3_h})
    res1 = run_bass_kernel_spmd(nc1, in_maps1, core_ids=CORE_IDS)
    parts = [r["qkvT"] for r in res1.results]          # each [3, 6, 128, 392]
    projT = np.concatenate(parts, axis=-1).reshape(3, D, S)  # [t, d, s]

    qh = projT[0].reshape(H, DK, S).transpose(0, 2, 1)[None]
    kh = projT[1].reshape(H, DK, S).transpose(0, 2, 1)[None]
    vh = projT[2].reshape(H, DK, S).transpose(0, 2, 1)[None]

    # ---------------- phase 2: attention (query-frame sharded) -------------
    khT_pad = np.zeros((H, DK, SP), f32)
    khT_pad[:, :, :S] = projT[1].reshape(H, DK, S)
    vh_pad = np.zeros((H, SP, VW), f32)
    vh_pad[:, :S, :DK] = vh[0]
    vh_pad[:, :S, DK] = 1.0
    vh_in = np.ascontiguousarray(
        vh_pad.reshape(H, NCH, P, VW).transpose(0, 2, 1, 3)
    ).reshape(H, P, NCH * VW)
    woT_in = np.ascontiguousarray(Wo.T.reshape(6, P, D).transpose(1, 0, 2))
    bo_in = np.ascontiguousarray(bo[None, :])
    ones_in = np.ones((1, QW), f32)
    frame = np.arange(SP) // NP
    qhT_all = projT[0].reshape(H, DK, S)

    in_maps2 = []
    for c in range(NCORE):
        rows = slice(QW * c, QW * (c + 1))
        fa, fb = 2 * c, 2 * c + 1
        minds_c = np.ascontiguousarray(
            np.stack([(frame > fa), (frame > fb)]).astype(f32)
        )
        qhT_c = np.zeros((H, CD, QW), f32)
        qhT_c[:, :DK, :] = qhT_all[:, :, rows]
        qhT_c[:, DK, :NP] = BIGNEG
        qhT_c[:, DK + 1, NP:] = BIGNEG
        in_maps2.append(
            {
                "khT": khT_pad,
                "minds": minds_c,
                "qhT": qhT_c,
                "vh": vh_in,
                "woT": woT_in,
                "bo": bo_in,
                "ones": ones_in,
            }
        )
    res2 = run_bass_kernel_spmd(nc2, in_maps2, core_ids=CORE_IDS)

    attention = np.empty((H, S, S), f32)
    out = np.empty((S, D), f32)
    for c in range(NCORE):
        rows = slice(QW * c, QW * (c + 1))
        attention[:, rows, :] = res2.results[c]["attn"][:, :S, :].transpose(0, 2, 1)
        out[rows] = res2.results[c]["outp"]

    return qh, kh, vh, attention[None], out[None]


# revision 7
# speedup vs baseline: 1.3676x; 1.3676x over previous
"""Trainium2 Bass kernel for block-causal (frame) multi-head attention.

Model (reference returns a 5-tuple):
    qh, kh, vh = per-head projections        [1, 12, 3136, 64]
    attention  = softmaxed masked scores      [1, 12, 3136, 3136]
    out        = output projection            [1, 3136, 768]

Strategy (8 NeuronCores, SPMD, uniform program, per-core data):
  Phase 1  (row-sharded QKV projection): core c computes q/k/v projections
      for its 392 query rows (frames 2c, 2c+1) in transposed layout
      [768, 392] via float32r matmuls.  Host gathers.
  Phase 2  (query-sharded attention): core c handles query frames 2c,2c+1
      for all 12 heads over all (padded) 3200 keys.  Scores are computed
      k-major (keys on partitions) with the block-causal mask folded into
      the matmul as 2 extra contraction rows (indicator x -BIG block).
      exp on ACT (scale=1/8 folded), denominator via an appended ones
      column in V, AV accumulated in PSUM, normalize on DVE, attention
      written k-major (host assembles the transpose).  Output projection
      + bias on-device.
"""

import numpy as np

import concourse.bacc as bacc
import concourse.mybir as mybir
import concourse.tile as tile
from concourse.bass_utils import run_bass_kernel_spmd

H = 12
D = 768
DK = 64
NP = 196
NF = 16
S = 3136
P = 128
NCORE = 8
QW = S // NCORE          # 392 query rows per core
SP = 3200                # keys padded to 25*128
NCH = SP // P            # 25 key chunks
CD = DK + 2              # contraction rows incl. 2 mask rows
VW = DK + 1              # v width incl. ones column
BIGNEG = np.float32(-60000.0)

F32 = mybir.dt.float32
F32R = mybir.dt.float32r
AF = mybir.ActivationFunctionType
CORE_IDS = list(range(NCORE))

_NC_CACHE = {}


def _build_phase1():
    nc = bacc.Bacc("TRN2", target_bir_lowering=False)
    xT = nc.dram_tensor("xT", [3, P, 6, QW], F32, kind="ExternalInput")
    wT = nc.dram_tensor("wT", [3, P, 6, D], F32, kind="ExternalInput")
    b3 = nc.dram_tensor("b3", [P, 18], F32, kind="ExternalInput")
    qkvT = nc.dram_tensor("qkvT", [3, 6, P, QW], F32, kind="ExternalOutput")

    with tile.TileContext(nc) as tc:
        with (
            tc.tile_pool(name="wp", bufs=1) as wp,
            tc.tile_pool(name="xp", bufs=1) as xp,
            tc.tile_pool(name="bp", bufs=1) as bp,
            tc.tile_pool(name="op", bufs=4) as op,
            tc.tile_pool(name="ps", bufs=4, space="PSUM") as ps,
        ):
            bt = bp.tile([P, 18], F32)
            nc.sync.dma_start(bt[:], b3[:])
            wts, xts = [], []
            for t in range(3):
                w = wp.tile([P, 6, D], F32R, tag=f"w{t}")
                nc.sync.dma_start(w[:], wT[t].bitcast(F32R))
                wts.append(w)
                x = xp.tile([P, 6, QW], F32R, tag=f"x{t}")
                nc.sync.dma_start(x[:], xT[t].bitcast(F32R))
                xts.append(x)
            for t in range(3):
                for dc in range(6):
                    pt = ps.tile([P, QW], F32)
                    for ec in range(6):
                        nc.tensor.matmul(
                            pt[:],
                            wts[t][:, ec, dc * P : (dc + 1) * P],
                            xts[t][:, ec, :],
                            start=(ec == 0),
                            stop=(ec == 5),
                        )
                    ot = op.tile([P, QW], F32, tag="out")
                    nc.scalar.activation(
                        ot[:],
                        pt[:],
                        AF.Identity,
                        bias=bt[:, t * 6 + dc : t * 6 + dc + 1],
                    )
                    nc.sync.dma_start(qkvT[t, dc], ot[:])
    nc.compile()
    return nc


def _build_phase2():
    nc = bacc.Bacc("TRN2", target_bir_lowering=False)
    khT = nc.dram_tensor("khT", [H, DK, SP], F32, kind="ExternalInput")
    minds = nc.dram_tensor("minds", [2, SP], F32, kind="ExternalInput")
    qhT = nc.dram_tensor("qhT", [H, CD, QW], F32, kind="ExternalInput")
    vh = nc.dram_tensor("vh", [H, P, NCH * VW], F32, kind="ExternalInput")
    woT = nc.dram_tensor("woT", [P, 6, D], F32, kind="ExternalInput")
    bo = nc.dram_tensor("bo", [1, D], F32, kind="ExternalInput")
    ones = nc.dram_tensor("ones", [1, QW], F32, kind="ExternalInput")
    attn = nc.dram_tensor("attn", [H, SP, QW], mybir.dt.bfloat16, kind="ExternalOutput")
    outp = nc.dram_tensor("outp", [QW, D], F32, kind="ExternalOutput")

    with tile.TileContext(nc) as tc:
        with (
            tc.tile_pool(name="kp", bufs=2) as kp,
            tc.tile_pool(name="qp", bufs=2) as qp,
            tc.tile_pool(name="vp", bufs=2) as vp,
            tc.tile_pool(name="ep", bufs=14) as ep,
            tc.tile_pool(name="ap", bufs=4) as ap,
            tc.tile_pool(name="cp", bufs=1) as cp,
            tc.tile_pool(name="ctp", bufs=2) as ctp,
            tc.tile_pool(name="rp", bufs=2) as rp,
            tc.tile_pool(name="wp", bufs=1) as wp,
            tc.tile_pool(name="bp", bufs=1) as bp,
            tc.tile_pool(name="op", bufs=3) as op,
            tc.tile_pool(name="ps_s", bufs=2, space="PSUM") as ps_s,
            tc.tile_pool(name="ps_o", bufs=1, space="PSUM") as ps_o,
            tc.tile_pool(name="ps_b", bufs=1, space="PSUM") as ps_b,
            tc.tile_pool(name="ps_f", bufs=2, space="PSUM") as ps_f,
        ):
            wo_t = wp.tile([P, 6, D], F32R)
            nc.sync.dma_start(wo_t[:], woT[:].bitcast(F32R))
            bo_t = bp.tile([1, D], F32R, tag="bo")
            nc.sync.dma_start(bo_t[:], bo[:].bitcast(F32R))
            ones_f = bp.tile([1, QW], F32, tag="ones_f")
            nc.sync.dma_start(ones_f[:], ones[:])
            ones_r = bp.tile([1, QW], F32R, tag="ones_r")
            nc.sync.dma_start(ones_r[:], ones[:].bitcast(F32R))

            concatT = cp.tile([P, 6, QW], F32R)

            for h in range(H):
                ka = kp.tile([CD, SP], F32R, tag="ka")
                nc.sync.dma_start(ka[0:DK, :], khT[h].bitcast(F32R))
                nc.sync.dma_start(ka[DK:CD, :], minds[:].bitcast(F32R))
                qa = qp.tile([CD, QW], F32R, tag="qa")
                nc.sync.dma_start(qa[:], qhT[h].bitcast(F32R))
                va = vp.tile([P, NCH * VW], F32R, tag="va")
                nc.sync.dma_start(va[:], vh[h].bitcast(F32R))

                po = ps_o.tile([VW, QW], F32, tag="po")
                exps = []
                # key chunks processed in groups of 2 so exp amortizes the
                # per-instruction ACT overhead; psum group tile is
                # bank-aligned [P, 2, 512]
                for g in range(13):
                    size = 2 if g < 12 else 1
                    st = ps_s.tile([P, 2, 512], F32, tag="st")
                    for k in range(size):
                        j = 2 * g + k
                        nc.tensor.matmul(
                            st[:, k, 0:QW], ka[:, j * P : (j + 1) * P], qa[:],
                            start=True, stop=True,
                        )
                    e = ep.tile([P, 2, QW], F32R, tag="exp")
                    nc.scalar.activation(
                        e[:, 0:size, :], st[:, 0:size, 0:QW], AF.Exp, scale=0.125
                    )
                    exps.append(e)
                    for k in range(size):
                        j = 2 * g + k
                        nc.tensor.matmul(
                            po[:], va[:, j * VW : (j + 1) * VW], e[:, k, :],
                            start=(j == 0), stop=(j == NCH - 1),
                        )

                rc = rp.tile([1, QW], F32, tag="rc")
                nc.vector.reciprocal(rc[:], po[DK : DK + 1, :])
                pb = ps_b.tile([P, QW], F32, tag="pb")
                nc.tensor.matmul(pb[:], ones_f[:, 0:P], rc[:], start=True, stop=True)
                rb = rp.tile([P, QW], F32, tag="rb")
                nc.scalar.copy(rb[:], pb[:])

                for g in range(13):
                    size = 2 if g < 12 else 1
                    at = ap.tile([P, 2, QW], mybir.dt.bfloat16, tag="attn")
                    nc.vector.tensor_tensor(
                        at[:, 0:size, :],
                        exps[g][:, 0:size, :].bitcast(F32),
                        rb[:, None, :].to_broadcast((P, size, QW)),
                        mybir.AluOpType.mult,
                    )
                    nc.sync.dma_start(
                        attn[h, g * 2 * P : g * 2 * P + size * P, :].rearrange(
                            "(k p) q -> p k q", p=P
                        ),
                        at[:, 0:size, :],
                    )

                cs = ctp.tile([DK, QW], F32, tag="cs")
                nc.vector.tensor_tensor(
                    cs[:], po[0:DK, :], rb[0:DK, :], mybir.AluOpType.mult
                )
                nc.scalar.copy(
                    concatT[(h % 2) * DK : (h % 2) * DK + DK, h // 2, :], cs[:]
                )

            for qc in range(4):
                outs = op.tile([98, D], F32, tag="outs")
                for half in range(2):
                    pf = ps_f.tile([98, 384], F32, tag="pf")
                    for ec in range(6):
                        nc.tensor.matmul(
                            pf[:],
                            concatT[:, ec, qc * 98 : (qc + 1) * 98],
                            wo_t[:, ec, half * 384 : (half + 1) * 384],
                            start=(ec == 0),
                            stop=False,
                            skip_group_check=True,
                        )
                    nc.tensor.matmul(
                        pf[:],
                        ones_r[:, qc * 98 : (qc + 1) * 98],
                        bo_t[:, half * 384 : (half + 1) * 384],
                        start=False,
                        stop=True,
                        skip_group_check=True,
                    )
                    nc.scalar.copy(outs[:, half * 384 : (half + 1) * 384], pf[:])
                nc.sync.dma_start(outp[qc * 98 : (qc + 1) * 98, :], outs[:])
    nc.compile()
    return nc


def _get_ncs():
    if "nc1" not in _NC_CACHE:
        _NC_CACHE["nc1"] = _build_phase1()
        _NC_CACHE["nc2"] = _build_phase2()
    return _NC_CACHE["nc1"], _NC_CACHE["nc2"]


def kernel(query, key, value, Wq, bq, Wk, bk, Wv, bv, Wo, bo):
    f32 = np.float32
    query = np.ascontiguousarray(np.asarray(query, f32))
    key = np.ascontiguousarray(np.asarray(key, f32))
    value = np.ascontiguousarray(np.asarray(value, f32))
    Wq, Wk, Wv, Wo = (np.asarray(w, f32) for w in (Wq, Wk, Wv, Wo))
    bq, bk, bv, bo = (np.asarray(b, f32) for b in (bq, bk, bv, bo))

    nc1, nc2 = _get_ncs()

    # ---------------- phase 1: QKV projection (row-sharded) ----------------
    wT_h = np.ascontiguousarray(
        np.stack([Wq.T, Wk.T, Wv.T]).reshape(3, 6, P, D).transpose(0, 2, 1, 3)
    )
    b3_h = np.ascontiguousarray(
        np.stack([bq, bk, bv]).reshape(3, 6, P).transpose(2, 0, 1).reshape(P, 18)
    )
    in_maps1 = []
    for c in range(NCORE):
        rows = slice(QW * c, QW * (c + 1))
        X = np.stack([query[0, rows], key[0, rows], value[0, rows]])
        xT_h = np.ascontiguousarray(
            X.transpose(0, 2, 1).reshape(3, 6, P, QW).transpose(0, 2, 1, 3)
        )
        in_maps1.append({"xT": xT_h, "wT": wT_h, "b3": b3_h})
    res1 = run_bass_kernel_spmd(nc1, in_maps1, core_ids=CORE_IDS)
    parts = [r["qkvT"] for r in res1.results]          # each [3, 6, 128, 392]
    projT = np.concatenate(parts, axis=-1).reshape(3, D, S)  # [t, d, s]

    qh = projT[0].reshape(H, DK, S).transpose(0, 2, 1)[None]
    kh = projT[1].reshape(H, DK, S).transpose(0, 2, 1)[None]
    vh = projT[2].reshape(H, DK, S).transpose(0, 2, 1)[None]

    # ---------------- phase 2: attention (query-frame sharded) -------------
    khT_pad = np.zeros((H, DK, SP), f32)
    khT_pad[:, :, :S] = projT[1].reshape(H, DK, S)
    vh_pad = np.zeros((H, SP, VW), f32)
    vh_pad[:, :S, :DK] = vh[0]
    vh_pad[:, :S, DK] = 1.0
    vh_in = np.ascontiguousarray(
        vh_pad.reshape(H, NCH, P, VW).transpose(0, 2, 1, 3)
    ).reshape(H, P, NCH * VW)
    woT_in = np.ascontiguousarray(Wo.T.reshape(6, P, D).transpose(1, 0, 2))
    bo_in = np.ascontiguousarray(bo[None, :])
    ones_in = np.ones((1, QW), f32)
    frame = np.arange(SP) // NP
    qhT_all = projT[0].reshape(H, DK, S)

    in_maps2 = []
    for c in range(NCORE):
        rows = slice(QW * c, QW * (c + 1))
        fa, fb = 2 * c, 2 * c + 1
        minds_c = np.ascontiguousarray(
            np.stack([(frame > fa), (frame > fb)]).astype(f32)
        )
        qhT_c = np.zeros((H, CD, QW), f32)
        qhT_c[:, :DK, :] = qhT_all[:, :, rows]
        qhT_c[:, DK, :NP] = BIGNEG
        qhT_c[:, DK + 1, NP:] = BIGNEG
        in_maps2.append(
            {
                "khT": khT_pad,
                "minds": minds_c,
                "qhT": qhT_c,
                "vh": vh_in,
                "woT": woT_in,
                "bo": bo_in,
                "ones": ones_in,
            }
        )
    res2 = run_bass_kernel_spmd(nc2, in_maps2, core_ids=CORE_IDS)

    attention = np.empty((H, S, S), f32)
    out = np.empty((S, D), f32)
    for c in range(NCORE):
        rows = slice(QW * c, QW * (c + 1))
        attention[:, rows, :] = (
            res2.results[c]["attn"][:, :S, :].astype(f32).transpose(0, 2, 1)
        )
        out[rows] = res2.results[c]["outp"]

    return qh, kh, vh, attention[None], out[None]


# revision 32
# speedup vs baseline: 1.7131x; 1.2526x over previous
"""Trainium2 Bass kernel for block-causal (frame) multi-head attention.

Model (reference returns a 5-tuple):
    qh, kh, vh = per-head projections        [1, 12, 3136, 64]
    attention  = softmaxed masked scores      [1, 12, 3136, 3136]
    out        = output projection            [1, 3136, 768]

Strategy (8 NeuronCores, SPMD, uniform program, per-core data):
  Phase 1  (row-sharded QKV projection): core c computes q/k/v projections
      for its 392 query rows (frames 2c, 2c+1) in transposed layout
      [768, 392] via float32r matmuls.  Host gathers.
  Phase 2  (query-sharded attention): core c handles query frames 2c,2c+1
      for all 12 heads over all (padded) 3200 keys.  Scores are computed
      k-major (keys on partitions) with the block-causal mask folded into
      the matmul as 2 extra contraction rows (indicator x -BIG block).
      exp on ACT (scale=1/8 folded), denominator via an appended ones
      column in V, AV accumulated in PSUM, normalize on DVE, attention
      written k-major (host assembles the transpose).  Output projection
      + bias on-device.
"""

import ml_dtypes
import numpy as np

import concourse.bacc as bacc
import concourse.mybir as mybir
import concourse.tile as tile
from concourse.bass_utils import run_bass_kernel_spmd

H = 12
D = 768
DK = 64
NP = 196
NF = 16
S = 3136
P = 128
NCORE = 8
QW = S // NCORE          # 392 query rows per core
SP = 3200                # keys padded to 25*128
NCH = SP // P            # 25 key chunks
CD = DK + 2              # contraction rows incl. 2 mask rows
VW = DK + 1              # v width incl. ones column
BIGNEG = np.float32(-60000.0)

F32 = mybir.dt.float32
F32R = mybir.dt.float32r
BF16 = mybir.dt.bfloat16
AF = mybir.ActivationFunctionType
CORE_IDS = list(range(NCORE))

_NC_CACHE = {}


def _build_phase1():
    nc = bacc.Bacc("TRN2", target_bir_lowering=False)
    xT = nc.dram_tensor("xT", [3, P, 6, QW], F32, kind="ExternalInput")
    wT = nc.dram_tensor("wT", [3, P, 6, D], F32, kind="ExternalInput")
    b3 = nc.dram_tensor("b3", [P, 18], F32, kind="ExternalInput")
    qkvT = nc.dram_tensor("qkvT", [3, 6, P, QW], F32, kind="ExternalOutput")

    with tile.TileContext(nc) as tc:
        with (
            tc.tile_pool(name="wp", bufs=1) as wp,
            tc.tile_pool(name="xp", bufs=1) as xp,
            tc.tile_pool(name="bp", bufs=1) as bp,
            tc.tile_pool(name="op", bufs=4) as op,
            tc.tile_pool(name="ps", bufs=4, space="PSUM") as ps,
        ):
            bt = bp.tile([P, 18], F32)
            nc.sync.dma_start(bt[:], b3[:])
            wts, xts = [], []
            for t in range(3):
                w = wp.tile([P, 6, D], F32R, tag=f"w{t}")
                x = xp.tile([P, 6, QW], F32R, tag=f"x{t}")
                for ec in range(6):
                    nc.sync.dma_start(x[:, ec, :], xT[t, :, ec, :].bitcast(F32R))
                    nc.sync.dma_start(w[:, ec, :], wT[t, :, ec, :].bitcast(F32R))
                wts.append(w)
                xts.append(x)
            for t in range(3):
                for dc in range(6):
                    pt = ps.tile([P, QW], F32)
                    for ec in range(6):
                        nc.tensor.matmul(
                            pt[:],
                            wts[t][:, ec, dc * P : (dc + 1) * P],
                            xts[t][:, ec, :],
                            start=(ec == 0),
                            stop=(ec == 5),
                        )
                    ot = op.tile([P, QW], F32, tag="out")
                    nc.scalar.activation(
                        ot[:],
                        pt[:],
                        AF.Identity,
                        bias=bt[:, t * 6 + dc : t * 6 + dc + 1],
                    )
                    nc.sync.dma_start(qkvT[t, dc], ot[:])
    nc.compile()
    return nc


def _build_phase2():
    nc = bacc.Bacc("TRN2", target_bir_lowering=False)
    khT = nc.dram_tensor("khT", [H, DK, SP], BF16, kind="ExternalInput")
    minds = nc.dram_tensor("minds", [2, SP], BF16, kind="ExternalInput")
    qhT = nc.dram_tensor("qhT", [H, CD, QW], BF16, kind="ExternalInput")
    vh = nc.dram_tensor("vh", [H, P, NCH * VW], BF16, kind="ExternalInput")
    woT = nc.dram_tensor("woT", [P, 6, D], BF16, kind="ExternalInput")
    bo = nc.dram_tensor("bo", [1, D], BF16, kind="ExternalInput")
    onesh = nc.dram_tensor("onesh", [1, QW], BF16, kind="ExternalInput")
    attn = nc.dram_tensor("attn", [H, SP, QW], mybir.dt.bfloat16, kind="ExternalOutput")
    outp = nc.dram_tensor("outp", [QW, D], F32, kind="ExternalOutput")

    with tile.TileContext(nc) as tc:
        with (
            tc.tile_pool(name="kp", bufs=2) as kp,
            tc.tile_pool(name="qp", bufs=2) as qp,
            tc.tile_pool(name="vp", bufs=2) as vp,
            tc.tile_pool(name="ep", bufs=10) as ep,
            tc.tile_pool(name="ap", bufs=2) as ap,
            tc.tile_pool(name="cp", bufs=1) as cp,
            tc.tile_pool(name="rp", bufs=2) as rp,
            tc.tile_pool(name="wp", bufs=1) as wp,
            tc.tile_pool(name="bp", bufs=1) as bp,
            tc.tile_pool(name="op", bufs=3) as op,
            tc.tile_pool(name="ps_s", bufs=2, space="PSUM") as ps_s,
            tc.tile_pool(name="ps_o", bufs=2, space="PSUM") as ps_o,
        ):
            concatT = cp.tile([P, 6, QW], BF16)

            def load_head(h):
                ka = kp.tile([CD, SP], BF16, tag="ka")
                nc.sync.dma_start(ka[0:DK, :], khT[h])
                nc.sync.dma_start(ka[DK:CD, :], minds[:])
                qa = qp.tile([CD, QW], BF16, tag="qa")
                nc.sync.dma_start(qa[:], qhT[h])
                va = vp.tile([P, NCH * VW], BF16, tag="va")
                nc.sync.dma_start(va[:], vh[h])
                return ka, qa, va

            loaded = load_head(0)
            for h in range(H):
                ka, qa, va = loaded

                po = ps_o.tile([VW, QW], F32, tag="po")
                exps = []
                # key chunks processed in groups of 3 so exp amortizes the
                # per-instruction ACT overhead; psum group tile is
                # bank-aligned [P, 3, 512]
                for g in range(9):
                    size = 3 if g < 8 else 1
                    st = ps_s.tile([P, 3, 512], F32, tag="st")
                    for k in range(size):
                        j = 3 * g + k
                        nc.tensor.matmul(
                            st[:, k, 0:QW], ka[:, j * P : (j + 1) * P], qa[:],
                            start=True, stop=True,
                        )
                    e = ep.tile([P, 3, QW], BF16, tag="exp")
                    nc.scalar.activation(
                        e[:, 0:size, :], st[:, 0:size, 0:QW], AF.Exp, scale=0.125
                    )
                    exps.append(e)
                    for k in range(size):
                        j = 3 * g + k
                        nc.tensor.matmul(
                            po[:], va[:, j * VW : (j + 1) * VW], e[:, k, :],
                            start=(j == 0), stop=(j == NCH - 1),
                        )

                if h + 1 < H:
                    loaded = load_head(h + 1)

                rc = rp.tile([1, QW], F32, tag="rc")
                nc.vector.reciprocal(rc[:], po[DK : DK + 1, :])
                rc_h = rp.tile([1, QW], BF16, tag="rc_h")
                nc.vector.tensor_copy(rc_h[:], rc[:])
                rb_f = rp.tile([P, QW], F32, tag="rb_f")
                nc.gpsimd.partition_broadcast(rb_f[:], rc[:])
                rb_h = rp.tile([P, QW], BF16, tag="rb_h")
                nc.gpsimd.partition_broadcast(rb_h[:], rc_h[:])

                abig = ap.tile([P, NCH, QW], BF16, tag="attn")
                for g in range(9):
                    size = 3 if g < 8 else 1
                    nc.vector.tensor_tensor(
                        abig[:, 3 * g : 3 * g + size, :],
                        exps[g][:, 0:size, :],
                        rb_h[:, None, :].to_broadcast((P, size, QW)),
                        mybir.AluOpType.mult,
                    )
                for w in range(5):
                    nc.sync.dma_start(
                        attn[h, w * 5 * P : (w + 1) * 5 * P, :].rearrange(
                            "(k p) q -> p k q", p=P
                        ),
                        abig[:, w * 5 : (w + 1) * 5, :],
                    )

                nc.vector.tensor_tensor(
                    concatT[(h % 2) * DK : (h % 2) * DK + DK, h // 2, :],
                    po[0:DK, :],
                    rb_f[0:DK, :],
                    mybir.AluOpType.mult,
                )

            wo_t = wp.tile([P, 6, D], BF16)
            nc.sync.dma_start(wo_t[:], woT[:])
            bo_t = bp.tile([1, D], BF16, tag="bo")
            nc.sync.dma_start(bo_t[:], bo[:])
            ones_b = bp.tile([1, QW], BF16, tag="ones_b")
            nc.sync.dma_start(ones_b[:], onesh[:])

            for qc in range(4):
                outs = op.tile([98, D], F32, tag="outs")
                for half in range(2):
                    pf = ps_o.tile([98, 384], F32, tag="po")
                    for ec in range(6):
                        nc.tensor.matmul(
                            pf[:],
                            concatT[:, ec, qc * 98 : (qc + 1) * 98],
                            wo_t[:, ec, half * 384 : (half + 1) * 384],
                            start=(ec == 0),
                            stop=False,
                            skip_group_check=True,
                        )
                    nc.tensor.matmul(
                        pf[:],
                        ones_b[:, qc * 98 : (qc + 1) * 98],
                        bo_t[:, half * 384 : (half + 1) * 384],
                        start=False,
                        stop=True,
                        skip_group_check=True,
                    )
                    nc.scalar.copy(outs[:, half * 384 : (half + 1) * 384], pf[:])
                nc.sync.dma_start(outp[qc * 98 : (qc + 1) * 98, :], outs[:])
    nc.compile()
    return nc


def _get_ncs():
    if "nc1" not in _NC_CACHE:
        _NC_CACHE["nc1"] = _build_phase1()
        _NC_CACHE["nc2"] = _build_phase2()
    return _NC_CACHE["nc1"], _NC_CACHE["nc2"]


def kernel(query, key, value, Wq, bq, Wk, bk, Wv, bv, Wo, bo):
    f32 = np.float32
    query = np.ascontiguousarray(np.asarray(query, f32))
    key = np.ascontiguousarray(np.asarray(key, f32))
    value = np.ascontiguousarray(np.asarray(value, f32))
    Wq, Wk, Wv, Wo = (np.asarray(w, f32) for w in (Wq, Wk, Wv, Wo))
    bq, bk, bv, bo = (np.asarray(b, f32) for b in (bq, bk, bv, bo))

    nc1, nc2 = _get_ncs()

    # ---------------- phase 1: QKV projection (row-sharded) ----------------
    wT_h = np.ascontiguousarray(
        np.stack([Wq.T, Wk.T, Wv.T]).reshape(3, 6, P, D).transpose(0, 2, 1, 3)
    )
    b3_h = np.ascontiguousarray(
        np.stack([bq, bk, bv]).reshape(3, 6, P).transpose(2, 0, 1).reshape(P, 18)
    )
    in_maps1 = []
    for c in range(NCORE):
        rows = slice(QW * c, QW * (c + 1))
        X = np.stack([query[0, rows], key[0, rows], value[0, rows]])
        xT_h = np.ascontiguousarray(
            X.transpose(0, 2, 1).reshape(3, 6, P, QW).transpose(0, 2, 1, 3)
        )
        in_maps1.append({"xT": xT_h, "wT": wT_h, "b3": b3_h})
    res1 = run_bass_kernel_spmd(nc1, in_maps1, core_ids=CORE_IDS)
    parts = [r["qkvT"] for r in res1.results]          # each [3, 6, 128, 392]
    projT = np.concatenate(parts, axis=-1).reshape(3, D, S)  # [t, d, s]

    qh = projT[0].reshape(H, DK, S).transpose(0, 2, 1)[None]
    kh = projT[1].reshape(H, DK, S).transpose(0, 2, 1)[None]
    vh = projT[2].reshape(H, DK, S).transpose(0, 2, 1)[None]

    # ---------------- phase 2: attention (query-frame sharded) -------------
    bf16 = ml_dtypes.bfloat16
    khT_pad = np.zeros((H, DK, SP), bf16)
    khT_pad[:, :, :S] = projT[1].reshape(H, DK, S)
    vh_pad = np.zeros((H, SP, VW), f32)
    vh_pad[:, :S, :DK] = vh[0]
    vh_pad[:, :S, DK] = 1.0
    vh_in = np.ascontiguousarray(
        vh_pad.reshape(H, NCH, P, VW).transpose(0, 2, 1, 3)
    ).reshape(H, P, NCH * VW).astype(bf16)
    woT_in = np.ascontiguousarray(Wo.T.reshape(6, P, D).transpose(1, 0, 2)).astype(bf16)
    bo_in = np.ascontiguousarray(bo[None, :]).astype(bf16)
    onesh_in = np.ones((1, QW), bf16)
    frame = np.arange(SP) // NP
    qhT_all = projT[0].reshape(H, DK, S)

    in_maps2 = []
    for c in range(NCORE):
        rows = slice(QW * c, QW * (c + 1))
        fa, fb = 2 * c, 2 * c + 1
        minds_c = np.ascontiguousarray(
            np.stack([(frame > fa), (frame > fb)]).astype(bf16)
        )
        qhT_c = np.zeros((H, CD, QW), bf16)
        qhT_c[:, :DK, :] = qhT_all[:, :, rows]
        qhT_c[:, DK, :NP] = BIGNEG
        qhT_c[:, DK + 1, NP:] = BIGNEG
        in_maps2.append(
            {
                "khT": khT_pad,
                "minds": minds_c,
                "qhT": qhT_c,
                "vh": vh_in,
                "woT": woT_in,
                "bo": bo_in,
                "onesh": onesh_in,
            }
        )
    res2 = run_bass_kernel_spmd(nc2, in_maps2, core_ids=CORE_IDS)

    attention = np.empty((H, S, S), f32)
    out = np.empty((S, D), f32)
    for c in range(NCORE):
        rows = slice(QW * c, QW * (c + 1))
        attention[:, rows, :] = (
            res2.results[c]["attn"][:, :S, :].astype(f32).transpose(0, 2, 1)
        )
        out[rows] = res2.results[c]["outp"]

    return qh, kh, vh, attention[None], out[None]


# revision 34
# speedup vs baseline: 1.8359x; 1.0717x over previous
"""Trainium2 Bass kernel for block-causal (frame) multi-head attention.

Model (reference returns a 5-tuple):
    qh, kh, vh = per-head projections        [1, 12, 3136, 64]
    attention  = softmaxed masked scores      [1, 12, 3136, 3136]
    out        = output projection            [1, 3136, 768]

Strategy (8 NeuronCores, SPMD, uniform program, per-core data):
  Phase 1  (row-sharded QKV projection): core c computes q/k/v projections
      for its 392 query rows (frames 2c, 2c+1) in transposed layout
      [768, 392] via float32r matmuls.  Host gathers.
  Phase 2  (query-sharded attention): core c handles query frames 2c,2c+1
      for all 12 heads over all (padded) 3200 keys.  Scores are computed
      k-major (keys on partitions) with the block-causal mask folded into
      the matmul as 2 extra contraction rows (indicator x -BIG block).
      exp on ACT (scale=1/8 folded), denominator via an appended ones
      column in V, AV accumulated in PSUM, normalize on DVE, attention
      written k-major (host assembles the transpose).  Output projection
      + bias on-device.
"""

import ml_dtypes
import numpy as np

import concourse.bacc as bacc
import concourse.mybir as mybir
import concourse.tile as tile
from concourse.bass_utils import run_bass_kernel_spmd

H = 12
D = 768
DK = 64
NP = 196
NF = 16
S = 3136
P = 128
NCORE = 8
QW = S // NCORE          # 392 query rows per core
SP = 3200                # keys padded to 25*128
NCH = SP // P            # 25 key chunks
CD = DK + 2              # contraction rows incl. 2 mask rows
VW = DK + 1              # v width incl. ones column
BIGNEG = np.float32(-30000.0)

F32 = mybir.dt.float32
F32R = mybir.dt.float32r
BF16 = mybir.dt.bfloat16
FP16 = mybir.dt.float16
AF = mybir.ActivationFunctionType
CORE_IDS = list(range(NCORE))

_NC_CACHE = {}


def _build_phase1():
    nc = bacc.Bacc("TRN2", target_bir_lowering=False)
    xT = nc.dram_tensor("xT", [3, P, 6, QW], F32, kind="ExternalInput")
    wT = nc.dram_tensor("wT", [3, P, 6, D], F32, kind="ExternalInput")
    b3 = nc.dram_tensor("b3", [P, 18], F32, kind="ExternalInput")
    qkvT = nc.dram_tensor("qkvT", [3, 6, P, QW], F32, kind="ExternalOutput")

    with tile.TileContext(nc) as tc:
        with (
            tc.tile_pool(name="wp", bufs=1) as wp,
            tc.tile_pool(name="xp", bufs=1) as xp,
            tc.tile_pool(name="bp", bufs=1) as bp,
            tc.tile_pool(name="op", bufs=4) as op,
            tc.tile_pool(name="ps", bufs=4, space="PSUM") as ps,
        ):
            bt = bp.tile([P, 18], F32)
            nc.sync.dma_start(bt[:], b3[:])
            wts, xts = [], []
            for t in range(3):
                w = wp.tile([P, 6, D], F32R, tag=f"w{t}")
                x = xp.tile([P, 6, QW], F32R, tag=f"x{t}")
                nc.sync.dma_start(x[:], xT[t].bitcast(F32R))
                nc.sync.dma_start(w[:], wT[t].bitcast(F32R))
                wts.append(w)
                xts.append(x)
            for t in range(3):
                for dc in range(6):
                    pt = ps.tile([P, QW], F32)
                    for ec in range(6):
                        nc.tensor.matmul(
                            pt[:],
                            wts[t][:, ec, dc * P : (dc + 1) * P],
                            xts[t][:, ec, :],
                            start=(ec == 0),
                            stop=(ec == 5),
                        )
                    ot = op.tile([P, QW], F32, tag="out")
                    nc.scalar.activation(
                        ot[:],
                        pt[:],
                        AF.Identity,
                        bias=bt[:, t * 6 + dc : t * 6 + dc + 1],
                    )
                    nc.sync.dma_start(qkvT[t, dc], ot[:])
    nc.compile()
    return nc


def _build_phase2():
    nc = bacc.Bacc("TRN2", target_bir_lowering=False)
    khT = nc.dram_tensor("khT", [H, DK, SP], FP16, kind="ExternalInput")
    minds = nc.dram_tensor("minds", [2, SP], FP16, kind="ExternalInput")
    qhT = nc.dram_tensor("qhT", [H, CD, QW], FP16, kind="ExternalInput")
    vh = nc.dram_tensor("vh", [H, P, NCH * VW], FP16, kind="ExternalInput")
    woT = nc.dram_tensor("woT", [P, 6, D], FP16, kind="ExternalInput")
    bo = nc.dram_tensor("bo", [1, D], FP16, kind="ExternalInput")
    onesh = nc.dram_tensor("onesh", [1, QW], FP16, kind="ExternalInput")
    attn = nc.dram_tensor("attn", [H, SP, QW], FP16, kind="ExternalOutput")
    outp = nc.dram_tensor("outp", [QW, D], F32, kind="ExternalOutput")

    with tile.TileContext(nc) as tc:
        with (
            tc.tile_pool(name="kp", bufs=2) as kp,
            tc.tile_pool(name="qp", bufs=2) as qp,
            tc.tile_pool(name="vp", bufs=2) as vp,
            tc.tile_pool(name="ep", bufs=10) as ep,
            tc.tile_pool(name="ap", bufs=2) as ap,
            tc.tile_pool(name="cp", bufs=1) as cp,
            tc.tile_pool(name="rp", bufs=2) as rp,
            tc.tile_pool(name="wp", bufs=1) as wp,
            tc.tile_pool(name="bp", bufs=1) as bp,
            tc.tile_pool(name="op", bufs=3) as op,
            tc.tile_pool(name="ps_s", bufs=2, space="PSUM") as ps_s,
            tc.tile_pool(name="ps_o", bufs=2, space="PSUM") as ps_o,
        ):
            concatT = cp.tile([P, 6, QW], FP16)

            def load_head(h):
                ka = kp.tile([CD, SP], FP16, tag="ka")
                nc.sync.dma_start(ka[0:DK, :], khT[h])
                nc.sync.dma_start(ka[DK:CD, :], minds[:])
                qa = qp.tile([CD, QW], FP16, tag="qa")
                nc.sync.dma_start(qa[:], qhT[h])
                va = vp.tile([P, NCH * VW], FP16, tag="va")
                nc.sync.dma_start(va[:], vh[h])
                return ka, qa, va

            loaded = load_head(0)
            for h in range(H):
                ka, qa, va = loaded

                po = ps_o.tile([VW, QW], F32, tag="po")
                exps = []
                # key chunks processed in groups of 3 so exp amortizes the
                # per-instruction ACT overhead; psum group tile is
                # bank-aligned [P, 3, 512]
                for g in range(9):
                    size = 3 if g < 8 else 1
                    st = ps_s.tile([P, 3, 512], F32, tag="st")
                    for k in range(size):
                        j = 3 * g + k
                        nc.tensor.matmul(
                            st[:, k, 0:QW], ka[:, j * P : (j + 1) * P], qa[:],
                            start=True, stop=True,
                        )
                    e = ep.tile([P, 3, QW], FP16, tag="exp")
                    nc.scalar.activation(
                        e[:, 0:size, :], st[:, 0:size, 0:QW], AF.Exp, scale=0.125
                    )
                    exps.append(e)
                    for k in range(size):
                        j = 3 * g + k
                        nc.tensor.matmul(
                            po[:], va[:, j * VW : (j + 1) * VW], e[:, k, :],
                            start=(j == 0), stop=(j == NCH - 1),
                        )

                if h + 1 < H:
                    loaded = load_head(h + 1)

                rc = rp.tile([1, QW], F32, tag="rc")
                nc.vector.reciprocal(rc[:], po[DK : DK + 1, :])
                rc_h = rp.tile([1, QW], FP16, tag="rc_h")
                nc.vector.tensor_copy(rc_h[:], rc[:])
                rb_f = rp.tile([P, QW], F32, tag="rb_f")
                nc.gpsimd.partition_broadcast(rb_f[:], rc[:])
                rb_h = rp.tile([P, QW], FP16, tag="rb_h")
                nc.gpsimd.partition_broadcast(rb_h[:], rc_h[:])

                abig = ap.tile([P, NCH, QW], FP16, tag="attn")
                for g in range(9):
                    size = 3 if g < 8 else 1
                    nc.vector.tensor_tensor(
                        abig[:, 3 * g : 3 * g + size, :],
                        exps[g][:, 0:size, :],
                        rb_h[:, None, :].to_broadcast((P, size, QW)),
                        mybir.AluOpType.mult,
                    )
                for w in range(5):
                    nc.sync.dma_start(
                        attn[h, w * 5 * P : (w + 1) * 5 * P, :].rearrange(
                            "(k p) q -> p k q", p=P
                        ),
                        abig[:, w * 5 : (w + 1) * 5, :],
                    )

                nc.vector.tensor_tensor(
                    concatT[(h % 2) * DK : (h % 2) * DK + DK, h // 2, :],
                    po[0:DK, :],
                    rb_f[0:DK, :],
                    mybir.AluOpType.mult,
                )

            wo_t = wp.tile([P, 6, D], FP16)
            nc.sync.dma_start(wo_t[:], woT[:])
            bo_t = bp.tile([1, D], FP16, tag="bo")
            nc.sync.dma_start(bo_t[:], bo[:])
            ones_b = bp.tile([1, QW], FP16, tag="ones_b")
            nc.sync.dma_start(ones_b[:], onesh[:])

            for qc in range(4):
                outs = op.tile([98, D], F32, tag="outs")
                for half in range(2):
                    pf = ps_o.tile([98, 384], F32, tag="po")
                    for ec in range(6):
                        nc.tensor.matmul(
                            pf[:],
                            concatT[:, ec, qc * 98 : (qc + 1) * 98],
                            wo_t[:, ec, half * 384 : (half + 1) * 384],
                            start=(ec == 0),
                            stop=False,
                            skip_group_check=True,
                        )
                    nc.tensor.matmul(
                        pf[:],
                        ones_b[:, qc * 98 : (qc + 1) * 98],
                        bo_t[:, half * 384 : (half + 1) * 384],
                        start=False,
                        stop=True,
                        skip_group_check=True,
                    )
                    nc.scalar.copy(outs[:, half * 384 : (half + 1) * 384], pf[:])
                nc.sync.dma_start(outp[qc * 98 : (qc + 1) * 98, :], outs[:])
    nc.compile()
    return nc


def _get_ncs():
    if "nc1" not in _NC_CACHE:
        _NC_CACHE["nc1"] = _build_phase1()
        _NC_CACHE["nc2"] = _build_phase2()
    return _NC_CACHE["nc1"], _NC_CACHE["nc2"]


def kernel(query, key, value, Wq, bq, Wk, bk, Wv, bv, Wo, bo):
    f32 = np.float32
    query = np.ascontiguousarray(np.asarray(query, f32))
    key = np.ascontiguousarray(np.asarray(key, f32))
    value = np.ascontiguousarray(np.asarray(value, f32))
    Wq, Wk, Wv, Wo = (np.asarray(w, f32) for w in (Wq, Wk, Wv, Wo))
    bq, bk, bv, bo = (np.asarray(b, f32) for b in (bq, bk, bv, bo))

    nc1, nc2 = _get_ncs()

    # ---------------- phase 1: QKV projection (row-sharded) ----------------
    wT_h = np.ascontiguousarray(
        np.stack([Wq.T, Wk.T, Wv.T]).reshape(3, 6, P, D).transpose(0, 2, 1, 3)
    )
    b3_h = np.ascontiguousarray(
        np.stack([bq, bk, bv]).reshape(3, 6, P).transpose(2, 0, 1).reshape(P, 18)
    )
    in_maps1 = []
    for c in range(NCORE):
        rows = slice(QW * c, QW * (c + 1))
        X = np.stack([query[0, rows], key[0, rows], value[0, rows]])
        xT_h = np.ascontiguousarray(
            X.transpose(0, 2, 1).reshape(3, 6, P, QW).transpose(0, 2, 1, 3)
        )
        in_maps1.append({"xT": xT_h, "wT": wT_h, "b3": b3_h})
    res1 = run_bass_kernel_spmd(nc1, in_maps1, core_ids=CORE_IDS)
    parts = [r["qkvT"] for r in res1.results]          # each [3, 6, 128, 392]
    projT = np.concatenate(parts, axis=-1).reshape(3, D, S)  # [t, d, s]

    qh = projT[0].reshape(H, DK, S).transpose(0, 2, 1)[None]
    kh = projT[1].reshape(H, DK, S).transpose(0, 2, 1)[None]
    vh = projT[2].reshape(H, DK, S).transpose(0, 2, 1)[None]

    # ---------------- phase 2: attention (query-frame sharded) -------------
    bf16 = np.float16
    khT_pad = np.zeros((H, DK, SP), bf16)
    khT_pad[:, :, :S] = projT[1].reshape(H, DK, S)
    vh_pad = np.zeros((H, SP, VW), f32)
    vh_pad[:, :S, :DK] = vh[0]
    vh_pad[:, :S, DK] = 1.0
    vh_in = np.ascontiguousarray(
        vh_pad.reshape(H, NCH, P, VW).transpose(0, 2, 1, 3)
    ).reshape(H, P, NCH * VW).astype(bf16)
    woT_in = np.ascontiguousarray(Wo.T.reshape(6, P, D).transpose(1, 0, 2)).astype(bf16)
    bo_in = np.ascontiguousarray(bo[None, :]).astype(bf16)
    onesh_in = np.ones((1, QW), bf16)
    frame = np.arange(SP) // NP
    qhT_all = projT[0].reshape(H, DK, S)

    in_maps2 = []
    for c in range(NCORE):
        rows = slice(QW * c, QW * (c + 1))
        fa, fb = 2 * c, 2 * c + 1
        minds_c = np.ascontiguousarray(
            np.stack([(frame > fa), (frame > fb)]).astype(bf16)
        )
        qhT_c = np.zeros((H, CD, QW), bf16)
        qhT_c[:, :DK, :] = qhT_all[:, :, rows]
        qhT_c[:, DK, :NP] = BIGNEG
        qhT_c[:, DK + 1, NP:] = BIGNEG
        in_maps2.append(
            {
                "khT": khT_pad,
                "minds": minds_c,
                "qhT": qhT_c,
                "vh": vh_in,
                "woT": woT_in,
                "bo": bo_in,
                "onesh": onesh_in,
            }
        )
    res2 = run_bass_kernel_spmd(nc2, in_maps2, core_ids=CORE_IDS)

    attention = np.empty((H, S, S), f32)
    out = np.empty((S, D), f32)
    for c in range(NCORE):
        rows = slice(QW * c, QW * (c + 1))
        attention[:, rows, :] = (
            res2.results[c]["attn"][:, :S, :].astype(f32).transpose(0, 2, 1)
        )
        out[rows] = res2.results[c]["outp"]

    return qh, kh, vh, attention[None], out[None]
